# revision 2
# baseline (speedup 1.0000x reference)
"""Locally-connected layer (3x3, stride 1) on 8 TRN2 NeuronCores.

Math (per reference): out[b,o,i,j] = sum_{c,kh,kw} x[b,c,i+kh,j+kw] * W[c,o,i,j,kh,kw] + bias[o,i,j]
  x: [128, 64, 32, 32] f32, W: [64, 64, 30, 30, 3, 3] f32, bias: [64, 30, 30] f32
  out: [128, 64, 30, 30] f32

Sharding: each core owns 4 output rows (cores 6,7 overlap rows 24-27/26-29 so all
cores run an identical program; host keeps rows 28-29 from core 7).

Per-core kernel: for each output position (i,j) accumulate 9 matmuls
  psum[b=128, o=64] += xT[c, b]^T @ W[c, o]  over taps (kh, kw)
with the x-pixel tile [c=64(+ones), b=128] as the PE-stationary operand shared
across all taps/rows that read pixel (h, w). Bias is added via a K=1 matmul of
an all-ones stationary row against the bias row, which also primes the PSUM
bank's has_written bits (start=True) for the whole bank.

Inputs are cast to bf16 and relaid on host so every device DMA is contiguous.
"""

import sys

for _p in ("/opt/trn_rl_repo",):
    if _p not in sys.path:
        sys.path.insert(0, _p)

import numpy as np
import ml_dtypes

import concourse.bass as bass
import concourse.tile as tile
from concourse import bacc, mybir
from concourse.bass_utils import run_bass_kernel_spmd

N_CORES = 8
B = 128
C = 64          # contracted channel dim (weight axis 0)
O = 64          # output channel dim (weight axis 1)
H = 32
W = 32
K = 3
OH = 30
OW = 30
R = 4           # output rows per core
H6 = R + K - 1  # input rows per core
ROW0 = [0, 4, 8, 12, 16, 20, 24, 26]  # first output row per core
BANDS = [(0, 8), (8, 8), (16, 8), (24, 6)]  # (j0, width) PSUM j-bands

XT_FREE = H6 * W * B            # xt free size: (h, w, b) = 24576
WT_PER_J = R * K * K * O        # 2304: (i, kh, kw, o)
WT_FREE = OW * WT_PER_J         # 69120

_BF16 = ml_dtypes.bfloat16


def build_nc(repeat: int = 1):
    """Build the per-core Bass program. `repeat` wraps the compute in a
    hardware loop (used only for timing)."""
    nc = bacc.Bacc("TRN2", target_bir_lowering=False, debug=False,
                   num_devices=N_CORES)
    xt_ap = nc.dram_tensor("xt", [C + 1, XT_FREE], mybir.dt.bfloat16,
                           kind="ExternalInput").ap()
    wt_ap = nc.dram_tensor("wt", [C + 1, WT_FREE], mybir.dt.bfloat16,
                           kind="ExternalInput").ap()
    out_ap = nc.dram_tensor("outp", [R, OW, B, O], mybir.dt.float32,
                            kind="ExternalOutput").ap()

    with tile.TileContext(nc) as tc:
        with (
            tc.tile_pool(name="xpool", bufs=1) as xpool,
            tc.tile_pool(name="wpool", bufs=2) as wpool,
            tc.tile_pool(name="ppool", bufs=8, space="PSUM") as ppool,
            tc.tile_pool(name="opool", bufs=4) as opool,
        ):
            xt_sb = xpool.tile([C + 1, XT_FREE], mybir.dt.bfloat16)
            # 8 chunked DMAs (split on h*w) to spread across queues
            n_x_dma = 8
            xchunk = XT_FREE // n_x_dma
            for q in range(n_x_dma):
                nc.sync.dma_start(xt_sb[:, q * xchunk:(q + 1) * xchunk],
                                  xt_ap[:, q * xchunk:(q + 1) * xchunk])
            # 3D views: [part, (h,w), b]
            xt3 = xt_sb[:].rearrange("p (f b) -> p f b", b=B)

            def body():
                for (j0, bw) in BANDS:
                    wt_sb = wpool.tile([C + 1, BANDS[0][1] * WT_PER_J],
                                       mybir.dt.bfloat16, tag="wt")
                    half = bw * WT_PER_J // 2
                    nc.sync.dma_start(wt_sb[:, :half],
                                      wt_ap[:, j0 * WT_PER_J:j0 * WT_PER_J + half])
                    nc.sync.dma_start(wt_sb[:, half:bw * WT_PER_J],
                                      wt_ap[:, j0 * WT_PER_J + half:(j0 + bw) * WT_PER_J])
                    wt3 = wt_sb[:].rearrange("p (j r) -> p j r", r=WT_PER_J)

                    ps = [ppool.tile([B, 512], mybir.dt.float32, tag="ps",
                                     name=f"ps{i}")
                          for i in range(R)]
                    # bias + has_written priming: ones[1,128]^T @ bias[1, bw*64]
                    ones_ap = xt3[C:C + 1, 0, :]
                    for i in range(R):
                        bias_rhs = wt3[C:C + 1, 0:bw, i * K * K * O:i * K * K * O + O]
                        nc.tensor.matmul(ps[i][:, 0:bw * O], ones_ap, bias_rhs,
                                         start=True, stop=False)
                    for h in range(H6):
                        for w in range(j0, min(j0 + bw + 2, W)):
                            lhs = xt3[0:C, h * W + w, :]
                            for kh in range(K):
                                i = h - kh
                                if i < 0 or i >= R:
                                    continue
                                for kw in range(K):
                                    j = w - kw
                                    if j < j0 or j >= j0 + bw:
                                        continue
                                    jl = j - j0
                                    off = jl * WT_PER_J + i * (K * K * O) \
                                        + kh * (K * O) + kw * O
                                    is_last = (kh == K - 1 and kw == K - 1
                                               and j == j0 + bw - 1)
                                    nc.tensor.matmul(
                                        ps[i][:, jl * O:(jl + 1) * O],
                                        lhs,
                                        wt_sb[0:C, off:off + O],
                                        start=False, stop=is_last)
                    for i in range(R):
                        ob = opool.tile([B, BANDS[0][1] * O], mybir.dt.float32,
                                        tag="ob")
                        nc.vector.tensor_copy(ob[:, 0:bw * O], ps[i][:, 0:bw * O])
                        dst = out_ap[i, j0:j0 + bw].transpose([1, 0, 2])
                        nc.sync.dma_start(
                            dst, ob[:, 0:bw * O].rearrange("p (j o) -> p j o", o=O))

            if repeat == 1:
                body()
            else:
                with tc.For_i(0, repeat, 1):
                    body()

    nc.compile()
    return nc


def prep_inputs(x: np.ndarray, weight: np.ndarray, bias: np.ndarray):
    """Host-side shard + relayout + bf16 cast. Returns in_maps for 8 cores."""
    in_maps = []
    for r0 in ROW0:
        # xt[c, h, w, b] (+ ones row) -> [65, 24576]
        xs = x[:, :, r0:r0 + H6, :]                      # [B, C, H6, W]
        xt = np.empty((C + 1, H6, W, B), dtype=_BF16)
        xt[:C] = xs.transpose(1, 2, 3, 0).astype(_BF16)
        xt[C] = np.ones((H6, W, B), dtype=_BF16)
        # wt[c, j, i, kh, kw, o] (+ bias row) -> [65, 69120]
        ws = weight[:, :, r0:r0 + R, :, :, :]            # [C, O, R, OW, K, K]
        wt = np.empty((C + 1, OW, R, K, K, O), dtype=_BF16)
        wt[:C] = ws.transpose(0, 3, 2, 4, 5, 1).astype(_BF16)
        wt[C] = 0
        wt[C, :, :, 0, 0, :] = bias[:, r0:r0 + R, :].transpose(2, 1, 0).astype(_BF16)
        in_maps.append({
            "xt": np.ascontiguousarray(xt.reshape(C + 1, XT_FREE)),
            "wt": np.ascontiguousarray(wt.reshape(C + 1, WT_FREE)),
        })
    return in_maps


def gather_output(results):
    out = np.empty((B, O, OH, OW), dtype=np.float32)
    for k, r0 in enumerate(ROW0):
        co = results[k]["outp"]                           # [R, OW, B, O]
        lo = 0 if k < 7 else 2                            # core 7: keep rows 28-29
        out[:, :, r0 + lo:r0 + R, :] = co[lo:].transpose(2, 3, 0, 1)
    return out


_NC_CACHE = {}


def kernel(x: np.ndarray, weight: np.ndarray, bias: np.ndarray) -> np.ndarray:
    if "nc" not in _NC_CACHE:
        _NC_CACHE["nc"] = build_nc()
    nc = _NC_CACHE["nc"]
    in_maps = prep_inputs(np.asarray(x), np.asarray(weight), np.asarray(bias))
    res = run_bass_kernel_spmd(nc, in_maps, core_ids=list(range(N_CORES)))
    return gather_output(res.results)


# revision 4
# speedup vs baseline: 1.0201x; 1.0201x over previous
"""Locally-connected layer (3x3, stride 1) on 8 TRN2 NeuronCores.

Math (per reference): out[b,o,i,j] = sum_{c,kh,kw} x[b,c,i+kh,j+kw] * W[c,o,i,j,kh,kw] + bias[o,i,j]
  x: [128, 64, 32, 32] f32, W: [64, 64, 30, 30, 3, 3] f32, bias: [64, 30, 30] f32
  out: [128, 64, 30, 30] f32

Sharding: each core owns 4 output rows (cores 6,7 overlap rows 24-27/26-29 so all
cores run an identical program; host keeps rows 28-29 from core 7).

Per-core kernel: for each output position (i,j) accumulate 9 matmuls
  psum[b=128, o=64] += xT[c, b]^T @ W[c, o]  over taps (kh, kw)
with the x-pixel tile [c=64(+ones), b=128] as the PE-stationary operand shared
across all taps/rows that read pixel (h, w). Bias is added via a K=1 matmul of
an all-ones stationary row against the bias row, which also primes the PSUM
bank's has_written bits (start=True) for the whole bank.

Inputs are cast to bf16 and relaid on host so every device DMA is contiguous.
"""

import sys

for _p in ("/opt/trn_rl_repo",):
    if _p not in sys.path:
        sys.path.insert(0, _p)

import numpy as np
import ml_dtypes

import concourse.bass as bass
import concourse.tile as tile
from concourse import bacc, mybir
from concourse.bass_utils import run_bass_kernel_spmd

N_CORES = 8
B = 128
C = 64          # contracted channel dim (weight axis 0)
O = 64          # output channel dim (weight axis 1)
H = 32
W = 32
K = 3
OH = 30
OW = 30
R = 4           # output rows per core
H6 = R + K - 1  # input rows per core
ROW0 = [0, 4, 8, 12, 16, 20, 24, 26]  # first output row per core
BANDS = [(0, 8), (8, 8), (16, 8), (24, 6)]  # (j0, width) PSUM j-bands

XT_FREE = H6 * W * B            # xt free size: (h, w, b) = 24576
WT_PER_J = R * K * K * O        # 2304: (i, kh, kw, o)
WT_FREE = OW * WT_PER_J         # 69120

_BF16 = ml_dtypes.bfloat16


def build_nc(repeat: int = 1):
    """Build the per-core Bass program. `repeat` wraps the compute in a
    hardware loop (used only for timing)."""
    nc = bacc.Bacc("TRN2", target_bir_lowering=False, debug=False,
                   num_devices=N_CORES)
    xt_ap = nc.dram_tensor("xt", [C + 1, XT_FREE], mybir.dt.bfloat16,
                           kind="ExternalInput").ap()
    wt_ap = nc.dram_tensor("wt", [C + 1, WT_FREE], mybir.dt.bfloat16,
                           kind="ExternalInput").ap()
    out_ap = nc.dram_tensor("outp", [R, OW, B, O], mybir.dt.float32,
                            kind="ExternalOutput").ap()

    with tile.TileContext(nc) as tc:
        with (
            tc.tile_pool(name="xpool", bufs=1) as xpool,
            tc.tile_pool(name="wpool", bufs=2) as wpool,
            tc.tile_pool(name="ppool", bufs=8, space="PSUM") as ppool,
            tc.tile_pool(name="opool", bufs=4) as opool,
        ):
            xt_sb = xpool.tile([C + 1, XT_FREE], mybir.dt.bfloat16)
            # 8 chunked DMAs (split on h*w) to spread across queues
            n_x_dma = 8
            xchunk = XT_FREE // n_x_dma
            for q in range(n_x_dma):
                nc.sync.dma_start(xt_sb[:, q * xchunk:(q + 1) * xchunk],
                                  xt_ap[:, q * xchunk:(q + 1) * xchunk])
            # 3D views: [part, (h,w), b]
            xt3 = xt_sb[:].rearrange("p (f b) -> p f b", b=B)

            def body():
                for (j0, bw) in BANDS:
                    wt_sb = wpool.tile([C + 1, BANDS[0][1] * WT_PER_J],
                                       mybir.dt.bfloat16, tag="wt")
                    half = bw * WT_PER_J // 2
                    nc.sync.dma_start(wt_sb[:, :half],
                                      wt_ap[:, j0 * WT_PER_J:j0 * WT_PER_J + half])
                    nc.sync.dma_start(wt_sb[:, half:bw * WT_PER_J],
                                      wt_ap[:, j0 * WT_PER_J + half:(j0 + bw) * WT_PER_J])
                    wt3 = wt_sb[:].rearrange("p (j r) -> p j r", r=WT_PER_J)

                    ps = [ppool.tile([B, 512], mybir.dt.float32, tag="ps",
                                     name=f"ps{i}")
                          for i in range(R)]
                    # bias + has_written priming: ones[1,128]^T @ bias[1, bw*64]
                    ones_ap = xt3[C:C + 1, 0, :]
                    for i in range(R):
                        bias_rhs = wt3[C:C + 1, 0:bw, i * K * K * O:i * K * K * O + O]
                        nc.tensor.matmul(ps[i][:, 0:bw * O], ones_ap, bias_rhs,
                                         start=True, stop=False)
                    for h in range(H6):
                        for w in range(j0, min(j0 + bw + 2, W)):
                            lhs = xt3[0:C, h * W + w, :]
                            for kh in range(K):
                                i = h - kh
                                if i < 0 or i >= R:
                                    continue
                                for kw in range(K):
                                    j = w - kw
                                    if j < j0 or j >= j0 + bw:
                                        continue
                                    jl = j - j0
                                    off = jl * WT_PER_J + i * (K * K * O) \
                                        + kh * (K * O) + kw * O
                                    is_last = (kh == K - 1 and kw == K - 1
                                               and j == j0 + bw - 1)
                                    nc.tensor.matmul(
                                        ps[i][:, jl * O:(jl + 1) * O],
                                        lhs,
                                        wt_sb[0:C, off:off + O],
                                        start=False, stop=is_last)
                    for i in range(R):
                        ob = opool.tile([B, BANDS[0][1] * O], mybir.dt.float32,
                                        tag="ob")
                        nc.vector.tensor_copy(ob[:, 0:bw * O], ps[i][:, 0:bw * O])
                        dst = out_ap[i, j0:j0 + bw].transpose([1, 0, 2])
                        nc.sync.dma_start(
                            dst, ob[:, 0:bw * O].rearrange("p (j o) -> p j o", o=O))

            if repeat == 1:
                body()
            else:
                with tc.For_i(0, repeat, 1):
                    body()

    nc.compile()
    dedup_ldweights(nc)
    return nc


def dedup_ldweights(nc):
    """Remove consecutive InstLdweights with identical weight APs from the PE
    stream (post-compile). The PE array keeps the stationary operand loaded
    across matmuls, so a reload of the same AP is pure overhead (~50-110ns
    each). Conservative: keeps any LDW that carries sync waits/updates or
    follows an intervening different LDW."""
    removed = 0
    for blk in nc.m.functions[0].blocks:
        insts = list(blk.instructions)
        has_pe = any(type(i).__name__ == "InstLdweights" for i in insts)
        if not has_pe:
            continue
        prev_key = None
        to_remove = []
        for inst in insts:
            nm = type(inst).__name__
            if nm == "InstLdweights":
                key = repr(inst.ins[0])
                si = inst.sync_info
                clean = not si or (not si.on_wait and not si.on_update)
                if key == prev_key and clean:
                    to_remove.append(inst)
                else:
                    prev_key = key
            elif nm == "InstMatmult":
                pass  # matmuls don't disturb the loaded stationary
            elif nm in ("InstEventSemaphore", "InstNop", "InstTensorLoad",
                        "InstTensorSave"):
                pass  # sequencer-only ops don't touch the PE array
            else:
                prev_key = None  # unknown PE-array effect: be safe
        for inst in to_remove:
            blk.instructions.remove(inst)
            removed += 1
    return removed


def prep_inputs(x: np.ndarray, weight: np.ndarray, bias: np.ndarray):
    """Host-side shard + relayout + bf16 cast. Returns in_maps for 8 cores."""
    in_maps = []
    for r0 in ROW0:
        # xt[c, h, w, b] (+ ones row) -> [65, 24576]
        xs = x[:, :, r0:r0 + H6, :]                      # [B, C, H6, W]
        xt = np.empty((C + 1, H6, W, B), dtype=_BF16)
        xt[:C] = xs.transpose(1, 2, 3, 0).astype(_BF16)
        xt[C] = np.ones((H6, W, B), dtype=_BF16)
        # wt[c, j, i, kh, kw, o] (+ bias row) -> [65, 69120]
        ws = weight[:, :, r0:r0 + R, :, :, :]            # [C, O, R, OW, K, K]
        wt = np.empty((C + 1, OW, R, K, K, O), dtype=_BF16)
        wt[:C] = ws.transpose(0, 3, 2, 4, 5, 1).astype(_BF16)
        wt[C] = 0
        wt[C, :, :, 0, 0, :] = bias[:, r0:r0 + R, :].transpose(2, 1, 0).astype(_BF16)
        in_maps.append({
            "xt": np.ascontiguousarray(xt.reshape(C + 1, XT_FREE)),
            "wt": np.ascontiguousarray(wt.reshape(C + 1, WT_FREE)),
        })
    return in_maps


def gather_output(results):
    out = np.empty((B, O, OH, OW), dtype=np.float32)
    for k, r0 in enumerate(ROW0):
        co = results[k]["outp"]                           # [R, OW, B, O]
        lo = 0 if k < 7 else 2                            # core 7: keep rows 28-29
        out[:, :, r0 + lo:r0 + R, :] = co[lo:].transpose(2, 3, 0, 1)
    return out


_NC_CACHE = {}


def kernel(x: np.ndarray, weight: np.ndarray, bias: np.ndarray) -> np.ndarray:
    if "nc" not in _NC_CACHE:
        _NC_CACHE["nc"] = build_nc()
    nc = _NC_CACHE["nc"]
    in_maps = prep_inputs(np.asarray(x), np.asarray(weight), np.asarray(bias))
    res = run_bass_kernel_spmd(nc, in_maps, core_ids=list(range(N_CORES)))
    return gather_output(res.results)
